# revision 8
# baseline (speedup 1.0000x reference)
"""Multi-head linear attention ('l1' attention) Bass kernel for 8 TRN2 NeuronCores.

Reference computation (fp32, batch 4, seq n=8192, d_model=1024, 16 heads x 64):
    q = softmax(x@Wq + bq, axis=dh); k = softmax(x@Wk + bk, axis=dh); v = x@Wv + bv
    k_sum = k.sum(rows);  d_inv = 1/((q*k_sum).sum(dh))
    ctx = k^T v (per head);  y = (q @ ctx) * d_inv + q;  out = y @ Wo + bo

Algebraic form used on-chip (s2 = sum_dh exp_q, s1 = sum_dh exp_q*k_sum):
    y = (exp_q @ ctx)/s1 + exp_q/s2
so the q-softmax normalization never needs a partition-axis divide.

Sharding: sequence-parallel. Rows (b*n = 32768) split into 8 contiguous chunks
of 4096; cores {2b, 2b+1} hold batch b, and ctx/k_sum partials are summed with
a 2-core AllReduce. Matmuls run in bf16 (fp32 accumulation); softmax and
normalization math stays fp32.

Layouts: x is fed pre-transposed (feature-major) from the host. k/v are
computed row-major (stationary = x^T chunk) so the dh-softmax is a free-dim
segmented reduce; q is computed feature-major (stationary = Wq chunk) so
exp_q @ ctx and the dh-sums contract over partitions on the PE.
"""

import contextlib
import sys

sys.path.insert(0, "/opt/trn_rl_repo")

import numpy as np
import ml_dtypes

import concourse.bass as bass
import concourse.mybir as mybir
import concourse.bacc as bacc
import concourse.tile as tile
from concourse.bass_utils import run_bass_kernel_spmd

BF16 = mybir.dt.bfloat16
F32 = mybir.dt.float32
F32R = mybir.dt.float32r
EXP = mybir.ActivationFunctionType.Exp
MUL = mybir.AluOpType.mult
ADD = mybir.AluOpType.add

D = 1024          # d_model
NCORES = 8
BLK = 512         # rows per block (moving-operand width)
BF = ml_dtypes.bfloat16


def build_attention(tc, R):
    """Emit the kernel for one core holding R rows (R % 512 == 0)."""
    nc = tc.nc
    NB = R // BLK
    groups = [[2 * i, 2 * i + 1] for i in range(NCORES // 2)]

    xT_d = nc.dram_tensor("xT", [D, R], BF16, kind="ExternalInput").ap()
    w_d = {
        n: nc.dram_tensor(n, [D, D], BF16, kind="ExternalInput").ap()
        for n in ("wq", "wk", "wv", "wo")
    }
    bq32_d = nc.dram_tensor("bq32", [D], F32, kind="ExternalInput").ap()
    bb_d = {
        n: nc.dram_tensor(n, [D], BF16, kind="ExternalInput").ap()
        for n in ("bkb", "bvb", "bob")
    }
    out_d = nc.dram_tensor("out", [R, D], F32, kind="ExternalOutput").ap()

    with (
        tc.tile_pool(name="cpool", bufs=1) as cpool,
        tc.tile_pool(name="xpool", bufs=16) as xpool,
        tc.tile_pool(name="ka", bufs=2) as ka,
        tc.tile_pool(name="bp", bufs=2) as bp,
        tc.tile_pool(name="eqp", bufs=3) as eqp,
        tc.tile_pool(name="ypool", bufs=2) as ypool,
        tc.tile_pool(name="dram", bufs=1, space="DRAM") as dram,
    ):
        # ---- persistent constants ----
        w_t = {}
        for n in ("wk", "wv", "wq", "wo"):
            w_t[n] = []
            for c in range(8):
                t = cpool.tile([128, D], BF16, tag=f"{n}{c}")
                nc.sync.dma_start(t[:], w_d[n][c * 128 : (c + 1) * 128, :])
                w_t[n].append(t)
        ones1 = cpool.tile([1, 128], BF16, tag="ones1")
        nc.vector.memset(ones1[:], 1.0)
        ones128 = cpool.tile([128, 1], BF16, tag="ones128")
        nc.vector.memset(ones128[:], 1.0)
        bias_sb = {}
        for n in ("bkb", "bvb", "bob"):
            t = cpool.tile([1, D], BF16, tag=f"{n}sb")
            nc.sync.dma_start(t[:], bb_d[n][None, :])
            bias_sb[n] = t
        bq_sb = cpool.tile([128, 8], F32, tag="bqsb")
        nc.sync.dma_start(bq_sb[:], bq32_d.rearrange("(f p) -> p f", p=128))

        # ================= Phase A: K/V projections, ctx & k_sum partials ====
        phaseA = contextlib.ExitStack()
        psKV = phaseA.enter_context(tc.tile_pool(name="psKV", bufs=1, space="PSUM"))
        psACC = phaseA.enter_context(tc.tile_pool(name="psACC", bufs=1, space="PSUM"))
        ctx_ps = psACC.tile([128, D], F32, tag="ctx")      # 2 banks
        ksum_ps = psACC.tile([128, 8], F32, tag="ksum")    # 1 bank
        ntiles = R // 128
        for b in range(NB):
            xt = []
            for c in range(8):
                t = xpool.tile([128, BLK], BF16, tag="xa")
                nc.sync.dma_start(
                    t[:], xT_d[c * 128 : (c + 1) * 128, b * BLK : (b + 1) * BLK]
                )
                xt.append(t)
            for j in range(4):
                t_idx = 4 * b + j
                first, last = t_idx == 0, t_idx == ntiles - 1
                k_ps = psKV.tile([128, D], F32, tag="kps")
                v_ps = psKV.tile([128, D], F32, tag="vps")
                for c in range(8):
                    st = xt[c][:, 128 * j : 128 * j + 128]
                    for h in range(2):
                        hs = slice(512 * h, 512 * h + 512)
                        nc.tensor.matmul(k_ps[:, hs], st, w_t["wk"][c][:, hs],
                                         start=(c == 0), stop=False)
                        nc.tensor.matmul(v_ps[:, hs], st, w_t["wv"][c][:, hs],
                                         start=(c == 0), stop=False)
                for h in range(2):
                    hs = slice(512 * h, 512 * h + 512)
                    nc.tensor.matmul(k_ps[:, hs], ones1[:], bias_sb["bkb"][:, hs],
                                     start=False, stop=True)
                    nc.tensor.matmul(v_ps[:, hs], ones1[:], bias_sb["bvb"][:, hs],
                                     start=False, stop=True)
                # k softmax over each head's 64 columns
                ke = ka.tile([128, D], F32, tag="ke")
                nc.scalar.activation(ke[:], k_ps[:], EXP)
                ks = ka.tile([128, 16], F32, tag="ks")
                nc.vector.reduce_sum(ks[:], ke[:].rearrange("p (n s) -> p n s", s=64),
                                     axis=mybir.AxisListType.X)
                kr = ka.tile([128, 16], F32, tag="kr")
                nc.vector.reciprocal(kr[:], ks[:])
                ksm = ka.tile([128, D], BF16, tag="ksm")
                nc.vector.tensor_tensor(
                    ksm[:].rearrange("p (n s) -> p n s", s=64),
                    ke[:].rearrange("p (n s) -> p n s", s=64),
                    kr[:].unsqueeze(2).broadcast_to([128, 16, 64]),
                    op=MUL,
                )
                vb = ka.tile([128, D], BF16, tag="vb")
                nc.scalar.copy(vb[:], v_ps[:])
                # ctx[d,e] += sum_rows ksm[r,d] * v[r,e]  (2 heads per 128-block)
                # ksum[d]  += sum_rows ksm[r,d]
                # start/stop once per PSUM bank: p=0..3 share a bank, p=4..7 the
                # other; start marks the whole bank pending-zero, later matmuls
                # in the same bank overwrite their own (pending-zero) bytes.
                for p in range(8):
                    ps = slice(128 * p, 128 * p + 128)
                    nc.tensor.matmul(ctx_ps[:, ps], ksm[:, ps], vb[:, ps],
                                     start=(first and p % 4 == 0),
                                     stop=(last and p % 4 == 3))
                    nc.tensor.matmul(ksum_ps[:, p : p + 1], ksm[:, ps], ones128[:],
                                     start=(first and p == 0),
                                     stop=(last and p == 7))

        # move partials to SBUF (DMA cannot read PSUM), zeroing the
        # cross-head garbage blocks of ctx before the reduce
        ctx_sb0 = cpool.tile([128, D], F32, tag="ctxsb0")
        nc.scalar.copy(ctx_sb0[:], ctx_ps[:])
        ksum_sb0 = cpool.tile([128, 8], F32, tag="ksumsb0")
        nc.vector.tensor_copy(ksum_sb0[:], ksum_ps[:])
        for p in range(8):
            nc.vector.memset(ctx_sb0[0:64, 128 * p + 64 : 128 * p + 128], 0.0)
            nc.vector.memset(ctx_sb0[64:128, 128 * p : 128 * p + 64], 0.0)

        phaseA.close()

        # ====== AllReduce ctx & k_sum across the 2 cores holding each batch ===
        bounce_in = dram.tile([128, 1032], F32)
        bounce_out = dram.tile([128, 1032], F32)
        nc.sync.dma_start(bounce_in[:, 0:1024], ctx_sb0[:])
        nc.sync.dma_start(bounce_in[:, 1024:1032], ksum_sb0[:])
        nc.gpsimd.collective_compute(
            "AllReduce",
            mybir.AluOpType.add,
            replica_groups=groups,
            ins=[bounce_in.opt()],
            outs=[bounce_out.opt()],
        )
        ctx_f32 = cpool.tile([128, D], F32, tag="ctxf")
        nc.sync.dma_start(ctx_f32[:], bounce_out[:, 0:1024])
        ksum_sb = cpool.tile([128, 8], F32, tag="ksumsb")
        nc.sync.dma_start(ksum_sb[:], bounce_out[:, 1024:1032])
        ctx_bf = cpool.tile([128, D], BF16, tag="ctxbf")
        nc.vector.tensor_copy(ctx_bf[:], ctx_f32[:])

        # ksel[:, 4f:4f+4] = [ksum_h1 | ksum_h2 | 1_h1 | 1_h2] for feature chunk f
        ksel = cpool.tile([128, 32], BF16, tag="ksel")
        nc.vector.memset(ksel[:], 0.0)
        for f in range(8):
            nc.vector.tensor_copy(ksel[0:64, 4 * f : 4 * f + 1], ksum_sb[0:64, f : f + 1])
            nc.vector.tensor_copy(ksel[64:128, 4 * f + 1 : 4 * f + 2],
                                  ksum_sb[64:128, f : f + 1])
            nc.vector.memset(ksel[0:64, 4 * f + 2 : 4 * f + 3], 1.0)
            nc.vector.memset(ksel[64:128, 4 * f + 3 : 4 * f + 4], 1.0)

        # head-block broadcast selectors: A from rows 0:2 (1/s1), B rows 2:4 (1/s2)
        sel_np = np.zeros((4, 256), np.float32)
        sel_np[0, 0:64] = 1.0
        sel_np[1, 64:128] = 1.0
        sel_np[2, 128:192] = 1.0
        sel_np[3, 192:256] = 1.0
        sel_dram = nc.inline_tensor(sel_np, name="selconst")
        sel = cpool.tile([4, 256], F32R, tag="sel")
        nc.gpsimd.dma_start(sel[:], sel_dram.ap())

        # ================= Phase B: Q path, y, output projection ==============
        phaseB = contextlib.ExitStack()
        psB1 = phaseB.enter_context(tc.tile_pool(name="psB1", bufs=1, space="PSUM"))
        psB2 = phaseB.enter_context(tc.tile_pool(name="psB2", bufs=2, space="PSUM"))
        for b in range(NB):
            xt = []
            for c in range(8):
                t = xpool.tile([128, BLK], BF16, tag="xa")
                nc.sync.dma_start(
                    t[:], xT_d[c * 128 : (c + 1) * 128, b * BLK : (b + 1) * BLK]
                )
                xt.append(t)
            yT = []
            for f in range(8):
                fs = slice(128 * f, 128 * f + 128)
                qT_ps = psB2.tile([128, BLK], F32, tag="qT")
                for c in range(8):
                    nc.tensor.matmul(qT_ps[:], w_t["wq"][c][:, fs], xt[c][:],
                                     start=(c == 0), stop=(c == 7))
                eq = eqp.tile([128, BLK], BF16, tag="eq")
                nc.scalar.activation(eq[:], qT_ps[:], EXP, bias=bq_sb[:, f : f + 1])
                s_ps = psB1.tile([4, BLK], F32, tag="s")
                nc.tensor.matmul(s_ps[:], ksel[:, 4 * f : 4 * f + 4], eq[:],
                                 start=True, stop=True)
                y1_ps = psB1.tile([128, BLK], F32, tag="y1")
                nc.tensor.matmul(y1_ps[:], ctx_bf[:, fs], eq[:], start=True, stop=True)
                rs = bp.tile([4, BLK], F32R, tag="rs")
                with nc.allow_low_precision(reason="f32r feed for broadcast matmul"):
                    nc.vector.reciprocal(rs[:], s_ps[:])
                A_ps = psB1.tile([128, BLK], F32, tag="Ab")
                nc.tensor.matmul(A_ps[:], sel[:, 0:128], rs[:],
                                 start=True, stop=True)
                B_ps = psB1.tile([128, BLK], F32, tag="Bb")
                nc.tensor.matmul(B_ps[:], sel[:, 128:256], rs[:],
                                 start=True, stop=True)
                y1_sb = bp.tile([128, BLK], F32, tag="y1s")
                nc.scalar.copy(y1_sb[:], y1_ps[:])
                t1 = bp.tile([128, BLK], F32, tag="t1")
                nc.vector.tensor_tensor(t1[:], y1_sb[:], A_ps[:], op=MUL)
                t2 = bp.tile([128, BLK], F32, tag="t2")
                nc.vector.tensor_tensor(t2[:], eq[:], B_ps[:], op=MUL)
                yt = ypool.tile([128, BLK], BF16, tag=f"yT{f}")
                nc.vector.tensor_tensor(yt[:], t1[:], t2[:], op=ADD)
                yT.append(yt)
            for h in range(2):
                hs = slice(512 * h, 512 * h + 512)
                for j in range(4):
                    o_ps = psB2.tile([128, BLK], F32, tag="ops")
                    for c in range(8):
                        nc.tensor.matmul(o_ps[:], yT[c][:, 128 * j : 128 * j + 128],
                                         w_t["wo"][c][:, hs],
                                         start=(c == 0), stop=False)
                    nc.tensor.matmul(o_ps[:], ones1[:], bias_sb["bob"][:, hs],
                                     start=False, stop=True)
                    o_sb = bp.tile([128, BLK], F32, tag="osb")
                    nc.scalar.copy(o_sb[:], o_ps[:])
                    r0 = BLK * b + 128 * j
                    nc.sync.dma_start(out_d[r0 : r0 + 128, hs], o_sb[:])
        phaseB.close()


_NC_CACHE = {}


def build_nc(R):
    if R in _NC_CACHE:
        return _NC_CACHE[R]
    nc = bacc.Bacc("TRN2", target_bir_lowering=False, debug=False,
                   num_devices=NCORES)
    with tile.TileContext(nc) as tc:
        build_attention(tc, R)
    nc.compile()
    _NC_CACHE[R] = nc
    return nc


def make_in_maps(x, Wq, bq, Wk, bk, Wv, bv, Wo, bo):
    """Host-side prep: cast to bf16, transpose x, shard rows over cores."""
    b, n, d = x.shape
    assert d == D
    flat = np.asarray(x, dtype=np.float32).reshape(-1, d)
    R = flat.shape[0] // NCORES
    xT = np.ascontiguousarray(flat.astype(BF).T)          # (D, total_rows)
    shared = {
        "wq": np.asarray(Wq, np.float32).astype(BF),
        "wk": np.asarray(Wk, np.float32).astype(BF),
        "wv": np.asarray(Wv, np.float32).astype(BF),
        "wo": np.asarray(Wo, np.float32).astype(BF),
        "bq32": np.asarray(bq, np.float32),
        "bkb": np.asarray(bk, np.float32).astype(BF),
        "bvb": np.asarray(bv, np.float32).astype(BF),
        "bob": np.asarray(bo, np.float32).astype(BF),
    }
    in_maps = [
        {"xT": np.ascontiguousarray(xT[:, c * R : (c + 1) * R]), **shared}
        for c in range(NCORES)
    ]
    return in_maps, R


def kernel(x, Wq, bq, Wk, bk, Wv, bv, Wo, bo, trace=False, **extra_kwargs):
    b, n, d = x.shape
    in_maps, R = make_in_maps(x, Wq, bq, Wk, bk, Wv, bv, Wo, bo)
    assert n % R == 0 or R % n == 0
    nc = build_nc(R)
    res = run_bass_kernel_spmd(nc, in_maps, core_ids=list(range(NCORES)),
                               trace=trace)
    out = np.concatenate([res.results[c]["out"] for c in range(NCORES)], axis=0)
    out = out.reshape(b, n, d)
    if trace:
        return out, res
    return out


# revision 21
# speedup vs baseline: 1.4065x; 1.4065x over previous
"""Multi-head linear attention ('l1' attention) Bass kernel for 8 TRN2 NeuronCores.

Reference computation (fp32, batch 4, seq n=8192, d_model=1024, 16 heads x 64):
    q = softmax(x@Wq + bq, axis=dh); k = softmax(x@Wk + bk, axis=dh); v = x@Wv + bv
    k_sum = k.sum(rows);  d_inv = 1/((q*k_sum).sum(dh))
    ctx = k^T v (per head);  y = (q @ ctx) * d_inv + q;  out = y @ Wo + bo

Algebraic form used on-chip (s2 = sum_dh exp_q, s1 = sum_dh exp_q*k_sum):
    y = (exp_q @ ctx)/s1 + exp_q/s2
so the q-softmax normalization never needs a partition-axis divide.

Sharding: sequence-parallel. Rows (b*n = 32768) split into 8 contiguous chunks
of 4096; cores {2b, 2b+1} hold batch b, and ctx/k_sum partials are summed with
a 2-core AllReduce. Matmuls run in bf16 (fp32 accumulation); softmax and
normalization math stays fp32.

Layouts: x is fed pre-transposed (feature-major) from the host. k/v are
computed row-major (stationary = x^T chunk) so the dh-softmax is a free-dim
segmented reduce; q is computed feature-major (stationary = Wq chunk) so
exp_q @ ctx and the dh-sums contract over partitions on the PE.
"""

import contextlib
import os
import sys

sys.path.insert(0, "/opt/trn_rl_repo")

import numpy as np
import ml_dtypes

import concourse.bass as bass
import concourse.mybir as mybir
import concourse.bacc as bacc
import concourse.tile as tile
from concourse.bass_utils import run_bass_kernel_spmd

BF16 = mybir.dt.bfloat16
F32 = mybir.dt.float32
F32R = mybir.dt.float32r
EXP = mybir.ActivationFunctionType.Exp
MUL = mybir.AluOpType.mult
ADD = mybir.AluOpType.add

D = 1024          # d_model
NCORES = 8
BLK = 512         # rows per block (moving-operand width)
BF = ml_dtypes.bfloat16


def build_attention(tc, R):
    """Emit the kernel for one core holding R rows (R % 512 == 0)."""
    nc = tc.nc
    NB = R // BLK
    groups = [[2 * i, 2 * i + 1] for i in range(NCORES // 2)]

    xT_d = nc.dram_tensor("xT", [D, R], BF16, kind="ExternalInput").ap()
    w_d = {
        n: nc.dram_tensor(n, [D, D], BF16, kind="ExternalInput").ap()
        for n in ("wq", "wk", "wv", "wo")
    }
    bq32_d = nc.dram_tensor("bq32", [D], F32, kind="ExternalInput").ap()
    bb_d = {
        n: nc.dram_tensor(n, [D], BF16, kind="ExternalInput").ap()
        for n in ("bkb", "bvb", "bob")
    }
    out_d = nc.dram_tensor("out", [R, D], F32, kind="ExternalOutput").ap()

    with (
        tc.tile_pool(name="cpool", bufs=1) as cpool,
        tc.tile_pool(name="xpool", bufs=16) as xpool,
        tc.tile_pool(name="ka", bufs=3) as ka,
        tc.tile_pool(name="bp", bufs=3) as bp,
        tc.tile_pool(name="eqp", bufs=4) as eqp,
        tc.tile_pool(name="ypool", bufs=4) as ypool,
        tc.tile_pool(name="dram", bufs=1, space="DRAM") as dram,
    ):
        # ---- persistent constants ----
        # (wq/wo tiles are allocated here but their DMAs are emitted later,
        # so phase A's first matmuls only wait on wk/wv + the first x block)
        w_t = {}
        for n in ("wk", "wv", "wq", "wo"):
            w_t[n] = [cpool.tile([128, D], BF16, tag=f"{n}{c}", name=f"{n}{c}")
                      for c in range(8)]

        def load_w(n):
            for c in range(8):
                nc.sync.dma_start(w_t[n][c][:], w_d[n][c * 128 : (c + 1) * 128, :])
        ones1 = cpool.tile([1, 128], BF16, tag="ones1")
        nc.vector.memset(ones1[:], 1.0)
        ones128 = cpool.tile([128, 1], BF16, tag="ones128")
        nc.vector.memset(ones128[:], 1.0)
        bias_sb = {}
        for n in ("bkb", "bvb", "bob"):
            t = cpool.tile([1, D], BF16, tag=f"{n}sb")
            nc.sync.dma_start(t[:], bb_d[n][None, :])
            bias_sb[n] = t
        bq_sb = cpool.tile([128, 8], F32, tag="bqsb")
        nc.sync.dma_start(bq_sb[:], bq32_d.rearrange("(f p) -> p f", p=128))

        def load_x(b):
            xt = []
            for c in range(8):
                t = xpool.tile([128, BLK], BF16, tag="xa")
                nc.sync.dma_start(
                    t[:], xT_d[c * 128 : (c + 1) * 128, b * BLK : (b + 1) * BLK]
                )
                xt.append(t)
            return xt

        # ================= Phase A: K/V projections, ctx & k_sum partials ====
        phaseA = contextlib.ExitStack()
        psKV = phaseA.enter_context(tc.tile_pool(name="psKV", bufs=2, space="PSUM"))
        psACC = phaseA.enter_context(tc.tile_pool(name="psACC", bufs=1, space="PSUM"))
        ctx_ps = psACC.tile([128, D], F32, tag="ctx")      # 2 banks
        ksum_ps = psACC.tile([128, 8], F32, tag="ksum")    # 1 bank
        ntiles = R // 128

        # ctx[d,e] += sum_rows ksm[r,d] * v[r,e]  (2 heads per 128-block)
        # ksum[d]  += sum_rows ksm[r,d]
        # start/stop once per PSUM bank: half h=0 (p=0..3) is one bank,
        # h=1 the other; start marks the whole bank pending-zero, later
        # matmuls in the bank overwrite their own (pending-zero) bytes.
        pipe = []

        def emit_ctx(t_idx, h, ksm_h, vb_h):
            first, last = t_idx == 0, t_idx == ntiles - 1
            for p4 in range(4):
                p = 4 * h + p4
                psl = slice(128 * p4, 128 * p4 + 128)
                nc.tensor.matmul(ctx_ps[:, 128 * p : 128 * p + 128],
                                 ksm_h[:, psl], vb_h[:, psl],
                                 start=(first and p4 == 0),
                                 stop=(last and p4 == 3))
                nc.tensor.matmul(ksum_ps[:, p : p + 1], ksm_h[:, psl], ones128[:],
                                 start=(first and p == 0),
                                 stop=(last and p == 7))

        first_xt = load_x(0)
        for c in range(8):
            nc.sync.dma_start(w_t["wk"][c][:], w_d["wk"][c * 128 : (c + 1) * 128, :])
            nc.sync.dma_start(w_t["wv"][c][:], w_d["wv"][c * 128 : (c + 1) * 128, :])
        for b in range(NB):
            xt = first_xt if b == 0 else load_x(b)
            for j in range(4):
                t_idx = 4 * b + j
                for h in range(2):
                    hs = slice(512 * h, 512 * h + 512)
                    k_ps = psKV.tile([128, 512], F32, tag="kps", name="k_ps")
                    v_ps = psKV.tile([128, 512], F32, tag="vps", name="v_ps")
                    for c in range(8):
                        st = xt[c][:, 128 * j : 128 * j + 128]
                        nc.tensor.matmul(k_ps[:], st, w_t["wk"][c][:, hs],
                                         start=(c == 0), stop=False)
                        nc.tensor.matmul(v_ps[:], st, w_t["wv"][c][:, hs],
                                         start=(c == 0), stop=False)
                    nc.tensor.matmul(k_ps[:], ones1[:], bias_sb["bkb"][:, hs],
                                     start=False, stop=True)
                    nc.tensor.matmul(v_ps[:], ones1[:], bias_sb["bvb"][:, hs],
                                     start=False, stop=True)
                    # k softmax over each head's 64 columns
                    ke = ka.tile([128, 512], F32, tag="ke", name="ke")
                    nc.scalar.activation(ke[:], k_ps[:], EXP)
                    ks = ka.tile([128, 8], F32, tag="ks", name="ks")
                    nc.vector.reduce_sum(ks[:],
                                         ke[:].rearrange("p (n s) -> p n s", s=64),
                                         axis=mybir.AxisListType.X)
                    kr = ka.tile([128, 8], F32, tag="kr", name="kr")
                    nc.vector.reciprocal(kr[:], ks[:])
                    ksm_h = ka.tile([128, 512], BF16, tag="ksm", name="ksm_h")
                    nc.vector.tensor_tensor(
                        ksm_h[:].rearrange("p (n s) -> p n s", s=64),
                        ke[:].rearrange("p (n s) -> p n s", s=64),
                        kr[:].unsqueeze(2).broadcast_to([128, 8, 64]),
                        op=MUL,
                    )
                    vb_h = ka.tile([128, 512], BF16, tag="vb", name="vb_h")
                    nc.scalar.copy(vb_h[:], v_ps[:])
                    # ctx/ksum matmuls run a half-tile behind the projections
                    # so the PE never waits on the current softmax chain.
                    pipe.append((t_idx, h, ksm_h, vb_h))
                    if len(pipe) > 2:
                        emit_ctx(*pipe.pop(0))

        while pipe:
            emit_ctx(*pipe.pop(0))

        # Pack only the useful diagonal 64x64 blocks of each head-pair ctx
        # block (plus ksum) into one compact buffer for the AllReduce.
        pack_sb = cpool.tile([128, 520], F32, tag="packsb")
        for p in range(8):
            nc.scalar.copy(pack_sb[0:64, 64 * p : 64 * p + 64],
                           ctx_ps[0:64, 128 * p : 128 * p + 64])
            nc.scalar.copy(pack_sb[64:128, 64 * p : 64 * p + 64],
                           ctx_ps[64:128, 128 * p + 64 : 128 * p + 128])
        nc.vector.tensor_copy(pack_sb[:, 512:520], ksum_ps[:])

        phaseA.close()

        # ====== AllReduce ctx & k_sum across the 2 cores holding each batch ===
        bounce_in = dram.tile([128, 520], F32)
        bounce_out = dram.tile([128, 520], F32)
        nc.sync.dma_start(bounce_in[:], pack_sb[:])
        nc.gpsimd.collective_compute(
            "AllReduce",
            mybir.AluOpType.add,
            replica_groups=groups,
            ins=[bounce_in.opt()],
            outs=[bounce_out.opt()],
        )
        unpack_sb = cpool.tile([128, 520], F32, tag="unpacksb")
        nc.sync.dma_start(unpack_sb[:], bounce_out[:])
        ksum_sb = unpack_sb[:, 512:520]
        # rebuild block-diagonal bf16 ctx (off-diagonal blocks zero)
        ctx_bf = cpool.tile([128, D], BF16, tag="ctxbf")
        nc.vector.memset(ctx_bf[:], 0.0)
        for p in range(8):
            nc.vector.tensor_copy(ctx_bf[0:64, 128 * p : 128 * p + 64],
                                  unpack_sb[0:64, 64 * p : 64 * p + 64])
            nc.vector.tensor_copy(ctx_bf[64:128, 128 * p + 64 : 128 * p + 128],
                                  unpack_sb[64:128, 64 * p : 64 * p + 64])

        # ksel[:, 4f:4f+4] = [ksum_h1 | ksum_h2 | 1_h1 | 1_h2] for feature chunk f
        ksel = cpool.tile([128, 32], BF16, tag="ksel")
        nc.vector.memset(ksel[:], 0.0)
        for f in range(8):
            nc.vector.tensor_copy(ksel[0:64, 4 * f : 4 * f + 1],
                                  unpack_sb[0:64, 512 + f : 513 + f])
            nc.vector.tensor_copy(ksel[64:128, 4 * f + 1 : 4 * f + 2],
                                  unpack_sb[64:128, 512 + f : 513 + f])
            nc.vector.memset(ksel[0:64, 4 * f + 2 : 4 * f + 3], 1.0)
            nc.vector.memset(ksel[64:128, 4 * f + 3 : 4 * f + 4], 1.0)

        # head-block broadcast selectors: A from rows 0:2 (1/s1), B rows 2:4 (1/s2)
        sel_np = np.zeros((4, 256), np.float32)
        sel_np[0, 0:64] = 1.0
        sel_np[1, 64:128] = 1.0
        sel_np[2, 128:192] = 1.0
        sel_np[3, 192:256] = 1.0
        sel_dram = nc.inline_tensor(sel_np, name="selconst")
        sel = cpool.tile([4, 256], F32R, tag="sel")
        nc.gpsimd.dma_start(sel[:], sel_dram.ap())

        # ================= Phase B: Q path, y, output projection ==============
        phaseB = contextlib.ExitStack()
        psB1 = phaseB.enter_context(tc.tile_pool(name="psB1", bufs=1, space="PSUM"))
        psB2 = phaseB.enter_context(tc.tile_pool(name="psB2", bufs=2, space="PSUM"))
        for b in range(NB):
            xt = []
            for c in range(8):
                t = xpool.tile([128, BLK], BF16, tag="xa")
                nc.sync.dma_start(
                    t[:], xT_d[c * 128 : (c + 1) * 128, b * BLK : (b + 1) * BLK]
                )
                xt.append(t)
            yT = []
            for f in range(8):
                fs = slice(128 * f, 128 * f + 128)
                qT_ps = psB2.tile([128, BLK], F32, tag="qT")
                for c in range(8):
                    nc.tensor.matmul(qT_ps[:], w_t["wq"][c][:, fs], xt[c][:],
                                     start=(c == 0), stop=(c == 7))
                eq = eqp.tile([128, BLK], BF16, tag="eq")
                nc.scalar.activation(eq[:], qT_ps[:], EXP, bias=bq_sb[:, f : f + 1])
                s_ps = psB1.tile([4, BLK], F32, tag="s")
                nc.tensor.matmul(s_ps[:], ksel[:, 4 * f : 4 * f + 4], eq[:],
                                 start=True, stop=True)
                y1_ps = psB1.tile([128, BLK], F32, tag="y1")
                nc.tensor.matmul(y1_ps[:], ctx_bf[:, fs], eq[:], start=True, stop=True)
                rs = bp.tile([4, BLK], F32R, tag="rs")
                with nc.allow_low_precision(reason="f32r feed for broadcast matmul"):
                    nc.vector.reciprocal(rs[:], s_ps[:])
                A_ps = psB1.tile([128, BLK], F32, tag="Ab")
                nc.tensor.matmul(A_ps[:], sel[:, 0:128], rs[:],
                                 start=True, stop=True)
                B_ps = psB1.tile([128, BLK], F32, tag="Bb")
                nc.tensor.matmul(B_ps[:], sel[:, 128:256], rs[:],
                                 start=True, stop=True)
                y1_sb = bp.tile([128, BLK], F32, tag="y1s")
                nc.scalar.copy(y1_sb[:], y1_ps[:])
                t1 = bp.tile([128, BLK], F32, tag="t1")
                nc.vector.tensor_tensor(t1[:], y1_sb[:], A_ps[:], op=MUL)
                t2 = bp.tile([128, BLK], F32, tag="t2")
                nc.vector.tensor_tensor(t2[:], eq[:], B_ps[:], op=MUL)
                yt = ypool.tile([128, BLK], BF16, tag=f"yT{f}")
                nc.vector.tensor_tensor(yt[:], t1[:], t2[:], op=ADD)
                yT.append(yt)
            for h in range(2):
                hs = slice(512 * h, 512 * h + 512)
                for j in range(4):
                    o_ps = psB2.tile([128, BLK], F32, tag="ops")
                    for c in range(8):
                        nc.tensor.matmul(o_ps[:], yT[c][:, 128 * j : 128 * j + 128],
                                         w_t["wo"][c][:, hs],
                                         start=(c == 0), stop=False)
                    nc.tensor.matmul(o_ps[:], ones1[:], bias_sb["bob"][:, hs],
                                     start=False, stop=True)
                    o_sb = bp.tile([128, BLK], F32, tag="osb")
                    nc.scalar.copy(o_sb[:], o_ps[:])
                    r0 = BLK * b + 128 * j
                    nc.sync.dma_start(out_d[r0 : r0 + 128, hs], o_sb[:])
        phaseB.close()


_NC_CACHE = {}


def build_nc(R):
    if R in _NC_CACHE:
        return _NC_CACHE[R]
    nc = bacc.Bacc("TRN2", target_bir_lowering=False, debug=False,
                   num_devices=NCORES)
    with tile.TileContext(nc) as tc:
        build_attention(tc, R)
    nc.compile()
    _NC_CACHE[R] = nc
    return nc


def make_in_maps(x, Wq, bq, Wk, bk, Wv, bv, Wo, bo):
    """Host-side prep: cast to bf16, transpose x, shard rows over cores."""
    b, n, d = x.shape
    assert d == D
    flat = np.asarray(x, dtype=np.float32).reshape(-1, d)
    R = flat.shape[0] // NCORES
    xT = np.ascontiguousarray(flat.astype(BF).T)          # (D, total_rows)
    shared = {
        "wq": np.asarray(Wq, np.float32).astype(BF),
        "wk": np.asarray(Wk, np.float32).astype(BF),
        "wv": np.asarray(Wv, np.float32).astype(BF),
        "wo": np.asarray(Wo, np.float32).astype(BF),
        "bq32": np.asarray(bq, np.float32),
        "bkb": np.asarray(bk, np.float32).astype(BF),
        "bvb": np.asarray(bv, np.float32).astype(BF),
        "bob": np.asarray(bo, np.float32).astype(BF),
    }
    in_maps = [
        {"xT": np.ascontiguousarray(xT[:, c * R : (c + 1) * R]), **shared}
        for c in range(NCORES)
    ]
    return in_maps, R


def kernel(x, Wq, bq, Wk, bk, Wv, bv, Wo, bo, trace=False, **extra_kwargs):
    b, n, d = x.shape
    in_maps, R = make_in_maps(x, Wq, bq, Wk, bk, Wv, bv, Wo, bo)
    assert n % R == 0 or R % n == 0
    nc = build_nc(R)
    res = run_bass_kernel_spmd(nc, in_maps, core_ids=list(range(NCORES)),
                               trace=trace)
    out = np.concatenate([res.results[c]["out"] for c in range(NCORES)], axis=0)
    out = out.reshape(b, n, d)
    if trace:
        return out, res
    return out
